# revision 1
# baseline (speedup 1.0000x reference)
"""Trainium2 Bass kernel for nn_EquivariantScalar (segment_reduce) — v5.

vs v6: fp8 reverted (error was 0.019, too close to the 2e-2 gate for no
speed gain) — h1/v2nb stay bf16 slices of the resident [F, 2, PAD] region
tensor. PE-gap fixes: tail matmuls split between sqrt phases and the start
of silu phases (covering the act-table-load transition stalls), and TWO
A-blocks of the next group pre-issued through each silu phase.
"""
import sys

if "/opt/trn_rl_repo" not in sys.path:
    sys.path.insert(0, "/opt/trn_rl_repo")

import numpy as np
import ml_dtypes

import concourse.bass as bass
import concourse.mybir as mybir
import concourse.tile as tile
from concourse.tile_rust import add_dep_helper as tile_rust_add_dep
from concourse.bass_utils import run_bass_kernel_spmd

F = 128
B = 256
BM = 64
N_NODES = 50000
N_CORES = 8
NPC = N_NODES // N_CORES
PAD = 6272
CHUNK = 512
BF16 = mybir.dt.bfloat16
FP32 = mybir.dt.float32
FP8 = mybir.dt.float8e4
AF = mybir.ActivationFunctionType
ALU = mybir.AluOpType

GROUPS = [6, 7]
WARMUP_MM = 12

_CACHE = {}

WNAMES = ["w2aT", "m1asT", "m1avT", "m2ahiT", "w1aT", "w2bT", "mfoldT",
          "m1bvT"]
NW = len(WNAMES)


def _chunks():
    out = []
    n0 = 0
    while n0 < PAD:
        w = min(CHUNK, PAD - n0)
        out.append((n0, w))
        n0 += w
    return out


def _build():
    nc = bass.Bass("TRN2", debug=False)

    sv_d = nc.dram_tensor("svT", (F, 4, PAD), BF16, kind="ExternalInput")
    m_d = nc.dram_tensor("mT", (F, PAD // F, BM), BF16, kind="ExternalInput")
    w_d = nc.dram_tensor("wpack", (F, NW + 1, F), BF16, kind="ExternalInput")
    b_d = nc.dram_tensor("bpack", (F, 4), FP32, kind="ExternalInput")
    y_d = nc.dram_tensor("y", (1, BM), FP32, kind="ExternalOutput")

    chunks = _chunks()
    nchunks = len(chunks)

    with nc.allow_low_precision(reason="bf16 intermediates are intentional"):
        with tile.TileContext(nc) as tc:
            with (
                tc.tile_pool(name="big", bufs=1) as big,
                tc.tile_pool(name="wk", bufs=2) as wk,
                tc.tile_pool(name="ps", bufs=1, space="PSUM") as ps,
            ):
                sv = big.tile([F, 4, PAD], BF16, name="sv_sb")
                wp = big.tile([F, NW + 1, F], BF16, name="wp_sb")
                bp = big.tile([F, 4], FP32, name="bp_sb")
                mt = big.tile([F, PAD // F, BM], BF16, name="mt_sb")

                nc.sync.dma_start(sv[:, :, 0:512], sv_d[:, :, 0:512])
                nc.sync.dma_start(wp[:], w_d[:])
                nc.sync.dma_start(bp[:], b_d[:])
                h8vb = big.tile([F, 2, PAD], BF16, name="hvb")
                nc.sync.dma_start(sv[:, :, 512:1536], sv_d[:, :, 512:1536])
                nc.sync.dma_start(mt[:], m_d[:])
                nc.sync.dma_start(sv[:, :, 1536:3584], sv_d[:, :, 1536:3584])
                nc.sync.dma_start(sv[:, :, 3584:PAD], sv_d[:, :, 3584:PAD])

                W = {n: wp[:, i, :] for i, n in enumerate(WNAMES)}
                WCOMB = wp[:, NW, 0:1]
                B1A = bp[:, 0:1]
                B2AHI = bp[:, 1:2]
                B1BE = bp[:, 2:3]
                ZERO = bp[:, 3:4]

                y_ps = ps.tile([1, BM], FP32, name="y_ps", tag="y", bufs=1)

                if WARMUP_MM:
                    pwarm = ps.tile([F, CHUNK], FP32, name="pwarm",
                                    tag="ph", bufs=3)
                    for i in range(WARMUP_MM):
                        nc.tensor.matmul(pwarm[:], wp[:, 0, :], wp[:, 0:4, :])

                last_act = [None]

                def act(*args, **kw):
                    inst = nc.scalar.activation(*args, **kw)
                    if last_act[0] is not None:
                        tile_rust_add_dep(inst.ins, last_act[0], sync=False,
                                          reason="act table-set ordering")
                    last_act[0] = inst.ins
                    return inst

                st = {}
                kctr = [0]

                def vin_of(ci, which, c):
                    n0, w = chunks[ci]
                    if which == 1:
                        return sv[:, 1 + c, n0:n0 + w]
                    return st[ci]["vo"][c][:, :w]

                def a_xy(ci, which):
                    # xy matmuls into a 2-bank tile; z matmul deferred
                    n0, w = chunks[ci]
                    sfx = "a" if which == 1 else "b"
                    wT = W["w2aT"] if which == 1 else W["w2bT"]
                    pxy = ps.tile([F, 2 * CHUNK], FP32,
                                  name=f"pxy{sfx}_{ci}", tag="pa", bufs=2)
                    nc.tensor.matmul(pxy[:, 0:w], wT, vin_of(ci, which, 0))
                    nc.tensor.matmul(pxy[:, CHUNK:CHUNK + w], wT,
                                     vin_of(ci, which, 1))
                    return pxy

                def a_sq_xy(ci, which, pxy):
                    n0, w = chunks[ci]
                    sfx = "a" if which == 1 else "b"
                    sq = wk.tile([F, 2 * CHUNK], BF16, name=f"sq{sfx}_{ci}",
                                 tag="sq", bufs=7)
                    if w == CHUNK:
                        act(sq[:], pxy[:], AF.Square, bias=ZERO)
                    else:
                        act(sq[:, :w], pxy[:, :w], AF.Square, bias=ZERO)
                        act(sq[:, CHUNK:CHUNK + w], pxy[:, CHUNK:CHUNK + w],
                            AF.Square, bias=ZERO)
                    return sq

                def a_block(entries, tails=()):
                    # entries: up to 2 SAME-KIND, chunk-adjacent (ci, which);
                    # shares a z tile and an nsq pair tile. Emits
                    # mms+squares+adds; returns pend for the lagged sqrt.
                    kctr[0] += 1
                    kk = kctr[0]
                    pz = ps.tile([F, 2 * CHUNK], FP32, name=f"pz_{kk}",
                                 tag="pa", bufs=2)
                    nsq2 = wk.tile([F, 2 * CHUNK], BF16, name=f"nsq_{kk}",
                                   tag="nsq", bufs=7)
                    v2n2 = wk.tile([F, 2 * CHUNK], BF16, name=f"v2n_{kk}",
                                   tag="v2n", bufs=9)
                    pxys = []
                    for idx, (ci, which) in enumerate(entries):
                        n0, w = chunks[ci]
                        pxy = a_xy(ci, which)
                        nc.tensor.matmul(
                            pz[:, idx * CHUNK:idx * CHUNK + w],
                            W["w2aT"] if which == 1 else W["w2bT"],
                            vin_of(ci, which, 2))
                        pxys.append(pxy)
                        if idx == 0:
                            for t in tails:
                                tail(t)
                    sqz = wk.tile([F, 2 * CHUNK], BF16, name=f"sqz_{kk}",
                                  tag="sqz", bufs=7)
                    sqs = []
                    for idx, (ci, which) in enumerate(entries):
                        sqs.append(a_sq_xy(ci, which, pxys[idx]))
                    ws = [chunks[ci][1] for ci, _ in entries]
                    if len(entries) == 2 and ws[0] == ws[1] == CHUNK:
                        act(sqz[:], pz[:], AF.Square, bias=ZERO)
                    else:
                        for idx in range(len(entries)):
                            act(sqz[:, idx * CHUNK:idx * CHUNK + ws[idx]],
                                pz[:, idx * CHUNK:idx * CHUNK + ws[idx]],
                                AF.Square, bias=ZERO)
                    for idx, (ci, which) in enumerate(entries):
                        w = ws[idx]
                        sq = sqs[idx]
                        t01 = wk.tile([F, CHUNK], BF16,
                                      name=f"t01_{kk}_{idx}", tag="t01",
                                      bufs=5)
                        nc.gpsimd.tensor_tensor(t01[:, :w], sq[:, :w],
                                                sq[:, CHUNK:CHUNK + w],
                                                ALU.add)
                        nc.vector.tensor_tensor(
                            nsq2[:, idx * CHUNK:idx * CHUNK + w], t01[:, :w],
                            sqz[:, idx * CHUNK:idx * CHUNK + w], ALU.add)
                    return (nsq2, v2n2, list(entries))

                def emit_sqrt(pend):
                    nsq2, v2n2, cur = pend
                    ws = [chunks[ci][1] for ci, _ in cur]
                    kind = cur[0][1]
                    if kind == 2:
                        # fp8 output into the resident region tensor
                        n0 = chunks[cur[0][0]][0]
                        wt = sum(ws)
                        if len(cur) == 2 and ws[0] == CHUNK:
                            act(h8vb[:, 1, n0:n0 + wt], nsq2[:, :wt],
                                AF.Sqrt, bias=ZERO)
                        else:
                            for idx, (ci, _) in enumerate(cur):
                                ni = chunks[ci][0]
                                act(h8vb[:, 1, ni:ni + ws[idx]],
                                    nsq2[:, idx * CHUNK:idx * CHUNK + ws[idx]],
                                    AF.Sqrt, bias=ZERO)
                        return
                    if len(cur) == 2 and ws[0] == ws[1] == CHUNK:
                        act(v2n2[:], nsq2[:], AF.Sqrt, bias=ZERO)
                    else:
                        for idx in range(len(cur)):
                            act(v2n2[:, idx * CHUNK:idx * CHUNK + ws[idx]],
                                nsq2[:, idx * CHUNK:idx * CHUNK + ws[idx]],
                                AF.Sqrt, bias=ZERO)
                    for idx, (ci, which) in enumerate(cur):
                        st.setdefault(ci, {})["v2na"] = \
                            v2n2[:, idx * CHUNK:(idx + 1) * CHUNK]

                def b1_mm(ci):
                    n0, w = chunks[ci]
                    d = st.setdefault(ci, {})
                    ph1 = ps.tile([F, CHUNK], FP32, name=f"ph1_{ci}",
                                  tag="ph", bufs=3)
                    nc.tensor.matmul(ph1[:, :w], W["m1asT"],
                                     sv[:, 0, n0:n0 + w],
                                     start=True, stop=False)
                    nc.tensor.matmul(ph1[:, :w], W["m1avT"],
                                     d["v2na"][:, :w], start=False, stop=True)
                    d["ph1"] = ph1

                def b1_head(ci):
                    n0, w = chunks[ci]
                    d = st[ci]
                    if "ph1" not in d:
                        b1_mm(ci)
                    ph1 = d.pop("ph1")
                    act(h8vb[:, 0, n0:n0 + w], ph1[:, :w], AF.Silu, bias=B1A)
                    d["h1"] = h8vb[:, 0, n0:n0 + w]

                def b1_tail(ci):
                    n0, w = chunks[ci]
                    d = st[ci]
                    h1 = d["h1"]
                    phi = ps.tile([F, CHUNK], FP32, name=f"phi_{ci}",
                                  tag="ph", bufs=3)
                    nc.tensor.matmul(phi[:, :w], W["m2ahiT"], h1[:, :w])
                    gate = wk.tile([F, CHUNK], BF16, name=f"gate_{ci}",
                                   tag="gate", bufs=3)
                    nc.vector.tensor_scalar_add(gate[:, :w], phi[:, :w],
                                                B2AHI)
                    vo = wk.tile([F, 3, CHUNK], BF16, name=f"vo_{ci}",
                                 tag="vo", bufs=9)
                    for c in range(3):
                        pv1 = ps.tile([F, CHUNK], FP32, name=f"pv1_{ci}_{c}",
                                      tag="ph", bufs=3)
                        nc.tensor.matmul(pv1[:, :w], W["w1aT"],
                                         sv[:, 1 + c, n0:n0 + w])
                        nc.vector.tensor_tensor(vo[:, c, :w], pv1[:, :w],
                                                gate[:, :w], ALU.mult)
                    d["vo"] = [vo[:, c, :] for c in range(3)]

                def phase_b2(ci):
                    n0, w = chunks[ci]
                    d = st[ci]
                    phb = ps.tile([F, CHUNK], FP32, name=f"phb_{ci}",
                                  tag="ph", bufs=3)
                    nc.tensor.matmul(phb[:, :w], W["mfoldT"],
                                     h8vb[:, 0, n0:n0 + w],
                                     start=True, stop=False)
                    nc.tensor.matmul(phb[:, :w], W["m1bvT"],
                                     h8vb[:, 1, n0:n0 + w],
                                     start=False, stop=True)
                    hb = wk.tile([F, CHUNK], BF16, name=f"hb_{ci}", tag="hb",
                                 bufs=6)
                    act(hb[:, :w], phb[:, :w], AF.Silu, bias=B1BE)
                    d["hb"] = hb

                def tail(ci):
                    n0, w = chunks[ci]
                    nsub = w // F
                    d = st[ci]
                    hb = d["hb"]
                    p_sf = ps.tile([F, CHUNK], FP32, name=f"psf_{ci}",
                                   tag="ph", bufs=3)
                    for j in range(nsub):
                        nc.tensor.matmul(p_sf[:, j:j + 1],
                                         hb[:, j * F:(j + 1) * F], WCOMB)
                    sf = wk.tile([F, 4], BF16, name=f"sf_{ci}", tag="sf",
                                 bufs=2)
                    nc.vector.tensor_copy(sf[:, :nsub], p_sf[:, :nsub])
                    J = n0 // F
                    for j in range(nsub):
                        first = ci == 0 and j == 0
                        last = ci == nchunks - 1 and j == nsub - 1
                        nc.tensor.matmul(y_ps[:], sf[:, j:j + 1],
                                         mt[:, J + j, :],
                                         start=first, stop=last,
                                         skip_group_check=True)
                    del st[ci]

                pre = {"a": [], "pend": []}

                def sqrt_phase(grp, prev, prev2, tails):
                    # pre["a"]/pre["pend"]: A-blocks already emitted during
                    # the previous silu phase with their sqrts still owed.
                    pends = list(pre["pend"])
                    pre["pend"] = []
                    done = set(pre["a"])
                    pre["a"] = []
                    ea = [(c, 1) for c in grp if (c, 1) not in done]
                    eb = [(p, 2) for p in prev if (p, 2) not in done]
                    blocks = []
                    ia = ib = 0
                    while ia < len(ea) or ib < len(eb):
                        if ia < len(ea):
                            blocks.append(ea[ia:ia + 2])
                            ia += 2
                        if ib < len(eb):
                            blocks.append(eb[ib:ib + 2])
                            ib += 2
                    ti = 0
                    for blk_e in blocks:
                        tl = tails[ti:ti + 1]
                        ti += 1
                        rec = a_block(blk_e, tl)
                        if pends:
                            emit_sqrt(pends.pop(0))
                        pends.append(rec)
                    for t in tails[ti:]:
                        tail(t)
                    for rec in pends:
                        emit_sqrt(rec)
                    # pre-issue mlp1 matmuls for the first chunks of grp
                    for ci in grp[:2]:
                        b1_mm(ci)

                def silu_phase(grp, prev, nxt, tails):
                    # aq: next group's A1 blocks, spread one per iteration;
                    # bq: this group's first A2 blocks, emitted as soon as
                    # both chunks of a pair have been gated (b1_tail done)
                    aq = [[(c, 1) for c in nxt[i:i + 2]]
                          for i in range(0, len(nxt), 2)]
                    bq = [[(c, 2) for c in grp[i:i + 2]]
                          for i in range(0, min(len(grp), 4), 2)]
                    tailed = set()
                    ti = 0
                    pend = []
                    for k in range(max(len(grp), len(prev))):
                        if k < len(grp):
                            b1_head(grp[k])
                        if ti < len(tails):
                            tail(tails[ti])
                            ti += 1
                        for ci in pend:
                            b1_tail(ci)
                            tailed.add(ci)
                        pend = [grp[k]] if k < len(grp) else []
                        if k < len(prev):
                            phase_b2(prev[k])
                        if bq and all(c in tailed for c, _ in bq[0]):
                            ent = bq.pop(0)
                            pre["pend"].append(a_block(ent))
                            pre["a"] += ent
                        elif k >= 1 and aq:
                            ent = aq.pop(0)
                            pre["pend"].append(a_block(ent))
                            pre["a"] += ent
                    for ci in pend:
                        b1_tail(ci)
                        tailed.add(ci)
                    for t in tails[ti:]:
                        tail(t)
                    while bq and all(c in tailed for c, _ in bq[0]):
                        ent = bq.pop(0)
                        pre["pend"].append(a_block(ent))
                        pre["a"] += ent
                    while aq:
                        ent = aq.pop(0)
                        pre["pend"].append(a_block(ent))
                        pre["a"] += ent

                groups = []
                c0 = 0
                for s in GROUPS:
                    groups.append(list(range(c0, min(c0 + s, nchunks))))
                    c0 += s
                for gi, grp in enumerate(groups):
                    prev = groups[gi - 1] if gi > 0 else []
                    prev2 = groups[gi - 2] if gi > 1 else []
                    nxt = groups[gi + 1] if gi + 1 < len(groups) else []
                    n2 = len(prev2)
                    sqrt_phase(grp, prev, prev2, list(prev2[:(n2 + 1) // 2]))
                    silu_phase(grp, prev, nxt, list(prev2[(n2 + 1) // 2:]))
                last = groups[-1]
                prelast = groups[-2] if len(groups) > 1 else []
                n2 = len(prelast)
                sqrt_phase([], last, prelast, list(prelast[:(n2 + 1) // 2]))
                silu_phase([], last, [], list(prelast[(n2 + 1) // 2:]))
                for cj in last:
                    tail(cj)

                y_sb = wk.tile([1, BM], FP32, name="y_sb", tag="ysb")
                nc.vector.tensor_copy(y_sb[:], y_ps[:])
                nc.sync.dma_start(y_d[:], y_sb[:])

    _dedupe_ldweights(nc)
    _split_sync_waits_inline(nc, max_waits=1)
    return nc


def _dedupe_ldweights(nc):
    f = nc.m.functions[0]
    removed = 0
    for blk in f.blocks:
        new_insts = []
        last_sig = None
        pending_waits = []
        for inst in blk.instructions:
            tn = type(inst).__name__
            if getattr(inst, "engine", None) != mybir.EngineType.PE:
                new_insts.append(inst)
                continue
            if tn == "InstLdweights":
                ap = inst.ins[0]
                sig = (ap.memref, ap.offset, str(ap.ap), str(ap.dtype),
                       str(getattr(inst, "perf_mode", None)))
                if sig == last_sig:
                    si = inst.sync_info
                    if si is not None:
                        pending_waits.extend(si.on_wait or [])
                        assert not si.on_update
                    removed += 1
                    continue
                last_sig = sig
            elif tn == "InstMatmult":
                if getattr(inst, "is_transpose", False):
                    last_sig = None
            if pending_waits:
                si = inst.sync_info
                old_w = list(si.on_wait) if si and si.on_wait else []
                old_u = list(si.on_update) if si and si.on_update else []
                inst.sync_info = mybir.SyncInfo(
                    on_wait=pending_waits + old_w, on_update=old_u)
                pending_waits = []
            new_insts.append(inst)
        assert not pending_waits
        blk.instructions[:] = new_insts
    return removed


def _split_sync_waits_inline(nc, max_waits=1):
    f = nc.m.functions[0]
    counter = [0]
    for blk in f.blocks:
        new_insts = []
        for inst in blk.instructions:
            si = getattr(inst, "sync_info", None)
            waits = list(si.on_wait) if si and si.on_wait else []
            if len(waits) > max_waits:
                head, rest = waits[:-max_waits], waits[-max_waits:]
                for i in range(0, len(head), max_waits):
                    counter[0] += 1
                    nop = mybir.InstNoOp(
                        name=f"I-wsplit-{counter[0]}",
                        engine=inst.engine,
                        ins=[],
                        outs=[],
                        sync_info=mybir.SyncInfo(
                            on_wait=head[i:i + max_waits], on_update=[]),
                    )
                    new_insts.append(nop)
                inst.sync_info = mybir.SyncInfo(on_wait=rest,
                                                on_update=list(si.on_update))
            new_insts.append(inst)
        blk.instructions[:] = new_insts


def _get_nc():
    if "nc" not in _CACHE:
        _CACHE["nc"] = _build()
    return _CACHE["nc"]


def _prep_inputs(s, v, batch_mask, w1, w2, mlp_w1, mlp_b1, mlp_w2, mlp_b2,
                 out_w, out_b):
    bf16 = ml_dtypes.bfloat16
    s = np.asarray(s, np.float32)
    v = np.asarray(v, np.float32)
    batch_mask = np.asarray(batch_mask, np.float32)

    w1 = np.asarray(w1, np.float32)
    w2 = np.asarray(w2, np.float32)
    mlp_w1 = np.asarray(mlp_w1, np.float32)
    mlp_b1 = np.asarray(mlp_b1, np.float32)
    mlp_w2 = np.asarray(mlp_w2, np.float32)
    mlp_b2 = np.asarray(mlp_b2, np.float32)
    out_w = np.asarray(out_w, np.float32)
    out_b = np.asarray(out_b, np.float32)

    m1bs = mlp_w1[1][:, :F]
    wfold = m1bs @ mlp_w2[0][:F, :]
    b1b_eff = mlp_b1[1] + m1bs @ mlp_b2[0][:F]
    wcomb = out_w[0] @ mlp_w2[1][:F, :]
    bconst = float(out_w[0] @ mlp_b2[1][:F] + out_b[0])

    wmats = {
        "w2aT": w2[0].T, "m1asT": mlp_w1[0][:, :F].T,
        "m1avT": mlp_w1[0][:, F:].T, "m2ahiT": mlp_w2[0][F:, :].T,
        "w1aT": w1[0].T, "w2bT": w2[1].T, "mfoldT": wfold.T,
        "m1bvT": mlp_w1[1][:, F:].T,
    }
    wpack = np.zeros((F, NW + 1, F), np.float32)
    for i, n in enumerate(WNAMES):
        wpack[:, i, :] = wmats[n]
    wpack[:, NW, 0] = wcomb
    bpack = np.zeros((F, 4), np.float32)
    bpack[:, 0] = mlp_b1[0]
    bpack[:, 1] = mlp_b2[0][F:]
    bpack[:, 2] = b1b_eff

    shared = {
        "wpack": np.ascontiguousarray(wpack.astype(bf16)),
        "bpack": np.ascontiguousarray(bpack),
    }

    mask_nb = batch_mask[:, :, 0].T
    mol_of_atom = np.argmax(mask_nb, axis=1)
    in_maps = []
    mol_lo = []
    for k in range(N_CORES):
        lo, hi = k * NPC, (k + 1) * NPC
        m0 = int(mol_of_atom[lo])
        m1 = int(mol_of_atom[hi - 1])
        assert m1 - m0 + 1 <= BM, f"core {k}: {m1 - m0 + 1} molecules > {BM}"
        mol_lo.append(m0)
        sk = np.zeros((PAD, F), np.float32)
        sk[:NPC] = s[0, lo:hi]
        vk = np.zeros((PAD, 3, F), np.float32)
        vk[:NPC] = v[0, lo:hi]
        mk = np.zeros((PAD, BM), np.float32)
        msl = mask_nb[lo:hi, m0:min(m0 + BM, B)]
        mk[:NPC, :msl.shape[1]] = msl
        m = dict(shared)
        svk = np.empty((F, 4, PAD), np.float32)
        svk[:, 0, :] = sk.T
        svk[:, 1:4, :] = vk.transpose(2, 1, 0)
        m["svT"] = np.ascontiguousarray(svk.astype(bf16))
        m["mT"] = np.ascontiguousarray(
            mk.reshape(PAD // F, F, BM).transpose(1, 0, 2).astype(bf16))
        in_maps.append(m)
    cnt = batch_mask[:, :, 0].sum(axis=1)
    return in_maps, mol_lo, bconst, cnt


def run(inputs, trace=False, **kw):
    nc = _get_nc()
    in_maps, mol_lo, bconst, cnt = _prep_inputs(
        inputs["s"], inputs["v"], inputs["batch_mask"], inputs["w1"],
        inputs["w2"], inputs["mlp_w1"], inputs["mlp_b1"], inputs["mlp_w2"],
        inputs["mlp_b2"], inputs["out_w"], inputs["out_b"])
    res = run_bass_kernel_spmd(nc, in_maps, list(range(N_CORES)),
                               trace=trace, **kw)
    y = np.zeros(B, np.float64)
    for k in range(N_CORES):
        yk = res.results[k]["y"].astype(np.float64).reshape(BM)
        m0 = mol_lo[k]
        nb = min(BM, B - m0)
        y[m0:m0 + nb] += yk[:nb]
    y += np.float64(bconst) * cnt.astype(np.float64)
    return y.astype(np.float32).reshape(B, 1), res


def kernel(**inputs):
    y, _ = run(inputs)
    return y

